# revision 21
# baseline (speedup 1.0000x reference)
"""Trainium2 Bass kernel for nn_BaseAttention (sliding-window attention).

Full-input contract: kernel(x, Wqkv) -> [B, T, C] float32.

Sharding (8 cores): data-parallel over B (2) x tensor-parallel over head
groups (16 heads -> 4 groups of 4). Core c handles batch c//4, head group
c%4. Each core computes its QKV projection slice (768 of 3072 output rows)
and banded attention for its 4 heads; outputs are disjoint channel slices
of the final [B, T, C] tensor, so no collectives are needed.

All matmul inputs are bf16 (rel err ~8e-3 vs the 2e-2 gate); accumulation
stays fp32 in PSUM. Design notes (from trace analysis):
  - per-matmul tensor cost ~= out_rows * 0.42ns when dependencies are hot,
    so instruction shapes maximize the moving dim: projection q,k in
    transposed [d, t] layout (512-wide), v in [t, d] (256-wide).
  - attention is key-chunk-centric: for key chunk j, ONE matmul produces
    scores^T [key 128, query <=384] covering query blocks j-1..j+1; exp is
    batched over head pairs on the scalar engine; the two window-mask
    multiplies are batched over all 4 heads on the (otherwise idle) gpsimd
    engine; PV is query-centric, accumulating [q, d|ones] chunks in PSUM so
    the softmax denominator falls out of the same matmuls; the vector
    engine normalizes straight out of PSUM. No PE transposes anywhere.
  - host pre-arranges x/W so every DMA is per-partition contiguous; x
    arrives in t-slices so the first projection matmul starts ~3us after
    the DMA queues open.
"""

import sys

import numpy as np

if "/opt/trn_rl_repo" not in sys.path:
    sys.path.insert(0, "/opt/trn_rl_repo")

B, T, C = 2, 2048, 1024
HEADS = 16
D = C // HEADS  # 64
WINDOW = 128
N_CORES = 8
HPC = HEADS // 4  # heads per core (4)
OPC = 3 * HPC * D  # projection output rows per core (768)

PDT_NAME = "bf16"

_PROGRAM_CACHE = {}

CC = C // 128  # 8 contraction chunks
TS = 512  # projection t-slice
NS = T // TS  # 4 slices
NB = T // 128  # 16 query / key blocks


def _build_program(pdt_name="bf16"):
    import concourse.mybir as mybir
    from concourse import bacc
    import concourse.tile as tile
    from contextlib import ExitStack

    f32 = mybir.dt.float32
    bf16 = mybir.dt.bfloat16
    Exp = mybir.ActivationFunctionType.Exp
    Ident = mybir.ActivationFunctionType.Identity

    nc = bacc.Bacc()
    # host pre-arranged, per-partition contiguous layouts
    xT_d = nc.declare_dram_parameter("xT", [128, NS * CC * TS], bf16, isOutput=False)
    wT_d = nc.declare_dram_parameter("wT", [128, CC * OPC], bf16, isOutput=False)
    msk_d = nc.declare_dram_parameter("msk", [128, 2, 2, 2, 128], bf16, isOutput=False)
    out_d = nc.declare_dram_parameter("out", [T, HPC * D], f32, isOutput=True)

    with ExitStack() as ctx:
        tc = ctx.enter_context(tile.TileContext(nc))
        const = ctx.enter_context(tc.tile_pool(name="const", bufs=1))
        lpool = ctx.enter_context(tc.tile_pool(name="lp", bufs=8))
        pj_ps = ctx.enter_context(tc.tile_pool(name="pjps", bufs=2, space="PSUM"))
        sc_ps = ctx.enter_context(tc.tile_pool(name="scps", bufs=2, space="PSUM"))
        ov_ps = ctx.enter_context(tc.tile_pool(name="ovps", bufs=2, space="PSUM"))

        w_sb = const.tile([128, CC, OPC], bf16)
        x_sb = const.tile([128, NS, CC, TS], bf16)
        msk_sb = const.tile([128, 2, 2, 2, 128], bf16)
        q_sb = const.tile([128, 2, T], bf16)
        k_sb = const.tile([128, 2, T], bf16)
        # v packed per (key block, head) with a trailing ones column: the PV
        # matmul emits the output block and the softmax denominator at once.
        v_sb = const.tile([128, NB, HPC, D + 1], bf16)
        # exp'd transposed scores, [p, mt, j, hh, 3*128]
        p_sb = const.tile([128, 2, NB, 2, 3 * 128], bf16)
        o_sb = const.tile([128, NB, HPC * D], f32)
        nc.vector.memset(v_sb[:, :, :, D:D + 1], 1.0)

        xT_r = xT_d.rearrange("p (s cc t) -> p s cc t", s=NS, cc=CC)
        wT_r = wT_d.rearrange("p (cc o) -> p cc o", cc=CC)
        # spread early-needed tensors over all three DMA-capable queues
        # (gpsimd/sync/scalar, each ~128GB/s), finely chunked up front so the
        # first projection chain starts as soon as w[cc0]+x[s0,cc0] land.
        nc.gpsimd.dma_start(out=w_sb[:, 0:1, :], in_=wT_r[:, 0:1, :])
        nc.sync.dma_start(out=x_sb[:, 0, 0:2, :], in_=xT_r[:, 0, 0:2, :])
        nc.scalar.dma_start(out=x_sb[:, 0, 4:6, :], in_=xT_r[:, 0, 4:6, :])
        nc.gpsimd.dma_start(out=w_sb[:, 1:2, :], in_=wT_r[:, 1:2, :])
        nc.sync.dma_start(out=x_sb[:, 0, 2:4, :], in_=xT_r[:, 0, 2:4, :])
        nc.scalar.dma_start(out=x_sb[:, 0, 6:8, :], in_=xT_r[:, 0, 6:8, :])
        nc.gpsimd.dma_start(out=w_sb[:, 2:4, :], in_=wT_r[:, 2:4, :])
        nc.sync.dma_start(out=w_sb[:, 4:6, :], in_=wT_r[:, 4:6, :])
        nc.scalar.dma_start(out=w_sb[:, 6:8, :], in_=wT_r[:, 6:8, :])
        for s in range(1, NS):
            nc.sync.dma_start(out=x_sb[:, s, 0:4, :], in_=xT_r[:, s, 0:4, :])
            nc.scalar.dma_start(out=x_sb[:, s, 4:8, :], in_=xT_r[:, s, 4:8, :])
        nc.gpsimd.dma_start(out=msk_sb, in_=msk_d[:, :, :, :, :])

        out_r = out_d.rearrange("(nb p) o -> p nb o", p=128)

        def emit_proj_m(s, m):
            # one q/k chain, transposed [o_part, t]; m-tiles: q0 q1 k0 k1
            ps = pj_ps.tile([128, TS], f32, tag="ps")
            for c in range(CC):
                nc.tensor.matmul(
                    ps,
                    lhsT=w_sb[:, c, m * 128:(m + 1) * 128],
                    rhs=x_sb[:, s, c, :],
                    start=(c == 0),
                    stop=(c == CC - 1),
                )
            dst = (q_sb if m < 2 else k_sb)[:, m % 2, s * TS:(s + 1) * TS]
            nc.scalar.copy(dst, ps)

        def emit_proj_v(s, t4):
            # one v chain, [t_part, o]
            pv = pj_ps.tile([128, TS], f32, tag="ps")
            for c in range(CC):
                nc.tensor.matmul(
                    pv[:, 0:D * HPC],
                    lhsT=x_sb[:, s, c, t4 * 128:(t4 + 1) * 128],
                    rhs=w_sb[:, c, 2 * D * HPC:3 * D * HPC],
                    start=(c == 0),
                    stop=(c == CC - 1),
                )
            tb = s * (TS // 128) + t4
            nc.vector.tensor_copy(
                v_sb[:, tb, :, 0:D],
                pv[:, 0:D * HPC].rearrange("p (h d) -> p h d", h=HPC),
            )

        def emit_qk(j):
            qlo = max(0, j - 1)
            qhi = min(NB - 1, j + 1)
            nq = (qhi - qlo + 1) * 128
            for mt in range(2):
                sct = sc_ps.tile([128, 2, TS], f32, tag="sc")
                for hh in range(2):
                    po = hh * 64
                    # scores^T chunk: [key j (part), query window (free)]
                    nc.tensor.matmul(
                        sct[:, hh, 0:nq],
                        lhsT=k_sb[po:po + 64, mt, j * 128:(j + 1) * 128],
                        rhs=q_sb[po:po + 64, mt, qlo * 128:(qhi + 1) * 128],
                        start=True,
                        stop=True,
                    )
                nc.scalar.activation(
                    p_sb[:, mt, j, :, 0:nq], sct[:, :, 0:nq], Exp
                )
            if j >= 1:  # query block j-1 sees chunk j as "next" (mask type 1)
                pm = p_sb[:, :, j, :, 0:128]
                nc.vector.tensor_mul(pm, pm, msk_sb[:, 1])
            if j <= NB - 2:  # query block j+1 sees chunk j as "prev" (type 0)
                c0 = (j + 1 - qlo) * 128
                pm = p_sb[:, :, j, :, c0:c0 + 128]
                nc.vector.tensor_mul(pm, pm, msk_sb[:, 0])

        def emit_pv(i):
            jbs = [jb for jb in (i - 1, i, i + 1) if 0 <= jb < NB]
            ov = ov_ps.tile([128, HPC, D + 1], f32, tag="ov")
            for h in range(HPC):
                mt, hh = divmod(h, 2)
                for n, j in enumerate(jbs):
                    ci = i - max(0, j - 1)
                    nc.tensor.matmul(
                        ov[:, h, :],
                        lhsT=p_sb[:, mt, j, hh, ci * 128:(ci + 1) * 128],
                        rhs=v_sb[:, j, h, :],
                        start=(n == 0),
                        stop=(n == len(jbs) - 1),
                    )
            r_t = lpool.tile([128, HPC], f32, tag="r")
            nc.vector.reciprocal(r_t, ov[:, :, D])
            for h in range(HPC):
                dst = o_sb[:, i, h * D:(h + 1) * D]
                if h < 2:
                    nc.vector.tensor_scalar_mul(
                        dst, ov[:, h, 0:D], r_t[:, h:h + 1]
                    )
                else:  # scalar engine: out = Identity(in * scale)
                    nc.scalar.activation(
                        dst, ov[:, h, 0:D], Ident, scale=r_t[:, h:h + 1]
                    )
            # batched early output DMAs; small late ones so the drain after
            # the last norm is ~one 128KB transfer, not a 10us dribble
            out_grp = {
                3: (nc.sync, 0, 4), 7: (nc.gpsimd, 4, 4), 11: (nc.sync, 8, 4),
                13: (nc.gpsimd, 12, 2), 14: (nc.sync, 14, 1), 15: (nc.gpsimd, 15, 1),
            }
            if i in out_grp:
                eng, b0, nb = out_grp[i]
                eng.dma_start(
                    out=out_r[:, b0:b0 + nb, :], in_=o_sb[:, b0:b0 + nb, :]
                )

        # fine interleave: attention units are woven between individual
        # projection chains so the in-order tensor queue never stalls on the
        # scalar exp chain — a not-yet-ready attention op would otherwise
        # block ready projection matmuls queued behind it.
        # s=0: prime the pipeline
        for m in range(4):
            emit_proj_m(0, m)
        emit_qk(0)
        emit_qk(1)
        emit_qk(2)
        emit_proj_v(0, 0)
        emit_proj_v(0, 1)
        emit_pv(0)
        emit_proj_v(0, 2)
        emit_pv(1)
        emit_proj_v(0, 3)
        for s in range(1, NS):
            emit_proj_m(s, 0)
            emit_proj_m(s, 1)
            emit_qk(4 * s - 1)
            emit_proj_m(s, 2)
            emit_proj_m(s, 3)
            emit_qk(4 * s)
            emit_qk(4 * s + 1)
            emit_qk(4 * s + 2)
            if s == NS - 1:
                emit_qk(4 * s + 3)
            emit_proj_v(s, 0)
            emit_pv(4 * s - 2)
            emit_proj_v(s, 1)
            emit_pv(4 * s - 1)
            emit_proj_v(s, 2)
            emit_pv(4 * s)
            emit_proj_v(s, 3)
            emit_pv(4 * s + 1)
        emit_pv(NB - 2)
        emit_pv(NB - 1)

    nc.compile()
    return nc


def _host_inputs(x, Wqkv):
    """Per-core input maps: shard batch x head-group, bf16, device layouts."""
    import ml_dtypes

    bf = ml_dtypes.bfloat16
    scale = float(D) ** -0.5
    r = np.arange(128, dtype=np.float32)[:, None]
    ci = np.arange(128, dtype=np.float32)[None, :]
    # type 0 (chunk is "prev" of the query block): allowed iff c <= r
    # type 1 (chunk is "next" of the query block): allowed iff c >= r
    msk2 = np.stack([(ci <= r), (ci >= r)], axis=1).astype(np.float32)
    msk = np.ascontiguousarray(
        np.broadcast_to(msk2[:, :, None, None, :], (128, 2, 2, 2, 128))
    ).astype(bf)

    x = np.asarray(x, dtype=np.float32)
    Wqkv = np.asarray(Wqkv, dtype=np.float32)
    # [p, s, cc, t] device layout, per-partition contiguous
    xT = [
        np.ascontiguousarray(
            x[b].T.reshape(CC, 128, NS, TS).transpose(1, 2, 0, 3).reshape(128, -1)
        ).astype(bf)
        for b in range(B)
    ]
    in_maps = []
    for core in range(N_CORES):
        b, hg = divmod(core, N_CORES // B)
        rows = slice(hg * HPC * D, (hg + 1) * HPC * D)
        wcat = np.concatenate(
            [
                Wqkv[0 * C:1 * C][rows] * scale,
                Wqkv[1 * C:2 * C][rows],
                Wqkv[2 * C:3 * C][rows],
            ],
            axis=0,
        )
        # [p, cc, o] device layout
        w = np.ascontiguousarray(
            wcat.T.reshape(CC, 128, OPC).transpose(1, 0, 2).reshape(128, -1)
        ).astype(bf)
        in_maps.append({"xT": xT[b], "wT": w, "msk": msk})
    return in_maps


def _gather(results):
    out = np.empty((B, T, C), dtype=np.float32)
    for core in range(N_CORES):
        b, hg = divmod(core, N_CORES // B)
        out[b, :, hg * HPC * D:(hg + 1) * HPC * D] = results[core]["out"]
    return out


def kernel(x, Wqkv):
    from concourse.bass_utils import run_bass_kernel_spmd

    key = PDT_NAME
    if key not in _PROGRAM_CACHE:
        _PROGRAM_CACHE[key] = _build_program(key)
    nc = _PROGRAM_CACHE[key]
    in_maps = _host_inputs(x, Wqkv)
    res = run_bass_kernel_spmd(nc, in_maps, list(range(N_CORES)))
    return _gather(res.results)


# revision 23
# speedup vs baseline: 1.0376x; 1.0376x over previous
"""Trainium2 Bass kernel for nn_BaseAttention (sliding-window attention).

Full-input contract: kernel(x, Wqkv) -> [B, T, C] float32.

Sharding (8 cores): data-parallel over B (2) x tensor-parallel over head
groups (16 heads -> 4 groups of 4). Core c handles batch c//4, head group
c%4. Each core computes its QKV projection slice (768 of 3072 output rows)
and banded attention for its 4 heads; outputs are disjoint channel slices
of the final [B, T, C] tensor, so no collectives are needed.

All matmul inputs are bf16 (rel err ~8e-3 vs the 2e-2 gate); accumulation
stays fp32 in PSUM. Design notes (from trace analysis):
  - per-matmul tensor cost ~= out_rows * 0.42ns when dependencies are hot,
    so instruction shapes maximize the moving dim: projection q,k in
    transposed [d, t] layout (512-wide), v in [t, d] (256-wide).
  - attention is key-chunk-centric: for key chunk j, ONE matmul produces
    scores^T [key 128, query <=384] covering query blocks j-1..j+1; exp is
    batched over head pairs on the scalar engine; the two window-mask
    multiplies are batched over all 4 heads on the (otherwise idle) gpsimd
    engine; PV is query-centric, accumulating [q, d|ones] chunks in PSUM so
    the softmax denominator falls out of the same matmuls; the vector
    engine normalizes straight out of PSUM. No PE transposes anywhere.
  - host pre-arranges x/W so every DMA is per-partition contiguous; x
    arrives in t-slices so the first projection matmul starts ~3us after
    the DMA queues open.
"""

import sys

import numpy as np

if "/opt/trn_rl_repo" not in sys.path:
    sys.path.insert(0, "/opt/trn_rl_repo")

B, T, C = 2, 2048, 1024
HEADS = 16
D = C // HEADS  # 64
WINDOW = 128
N_CORES = 8
HPC = HEADS // 4  # heads per core (4)
OPC = 3 * HPC * D  # projection output rows per core (768)

PDT_NAME = "bf16"

_PROGRAM_CACHE = {}

CC = C // 128  # 8 contraction chunks
TS = 512  # projection t-slice
NS = T // TS  # 4 slices
NB = T // 128  # 16 query / key blocks


def _build_program(pdt_name="bf16"):
    import concourse.mybir as mybir
    from concourse import bacc
    import concourse.tile as tile
    from contextlib import ExitStack

    f32 = mybir.dt.float32
    bf16 = mybir.dt.bfloat16
    Exp = mybir.ActivationFunctionType.Exp
    Ident = mybir.ActivationFunctionType.Identity

    nc = bacc.Bacc()
    # host pre-arranged, per-partition contiguous layouts
    xT_d = nc.declare_dram_parameter("xT", [128, NS * CC * TS], bf16, isOutput=False)
    wT_d = nc.declare_dram_parameter("wT", [128, CC * OPC], bf16, isOutput=False)
    msk_d = nc.declare_dram_parameter("msk", [128, 2, 2, 2, 128], bf16, isOutput=False)
    out_d = nc.declare_dram_parameter("out", [T, HPC * D], f32, isOutput=True)

    with ExitStack() as ctx:
        tc = ctx.enter_context(tile.TileContext(nc))
        const = ctx.enter_context(tc.tile_pool(name="const", bufs=1))
        lpool = ctx.enter_context(tc.tile_pool(name="lp", bufs=8))
        pj_ps = ctx.enter_context(tc.tile_pool(name="pjps", bufs=2, space="PSUM"))
        sc_ps = ctx.enter_context(tc.tile_pool(name="scps", bufs=2, space="PSUM"))
        ov_ps = ctx.enter_context(tc.tile_pool(name="ovps", bufs=2, space="PSUM"))

        w_sb = const.tile([128, CC, OPC], bf16)
        x_sb = const.tile([128, NS, CC, TS], bf16)
        msk_sb = const.tile([128, 2, 2, 2, 128], bf16)
        q_sb = const.tile([128, 2, T], bf16)
        k_sb = const.tile([128, 2, T], bf16)
        # v packed per (key block, head) with a trailing ones column: the PV
        # matmul emits the output block and the softmax denominator at once.
        v_sb = const.tile([128, NB, HPC, D + 1], bf16)
        # exp'd transposed scores, [p, mt, j, hh, 3*128]
        p_sb = const.tile([128, 2, NB, 2, 3 * 128], bf16)
        o_sb = const.tile([128, NB, HPC * D], f32)
        nc.vector.memset(v_sb[:, :, :, D:D + 1], 1.0)

        xT_r = xT_d.rearrange("p (s cc t) -> p s cc t", s=NS, cc=CC)
        wT_r = wT_d.rearrange("p (cc o) -> p cc o", cc=CC)
        # spread early-needed tensors over all three DMA-capable queues
        # (gpsimd/sync/scalar, each ~128GB/s), finely chunked up front so the
        # first projection chain starts as soon as w[cc0]+x[s0,cc0] land.
        nc.gpsimd.dma_start(out=w_sb[:, 0:1, :], in_=wT_r[:, 0:1, :])
        nc.sync.dma_start(out=x_sb[:, 0, 0:2, :], in_=xT_r[:, 0, 0:2, :])
        nc.scalar.dma_start(out=x_sb[:, 0, 4:6, :], in_=xT_r[:, 0, 4:6, :])
        nc.gpsimd.dma_start(out=w_sb[:, 1:2, :], in_=wT_r[:, 1:2, :])
        nc.sync.dma_start(out=x_sb[:, 0, 2:4, :], in_=xT_r[:, 0, 2:4, :])
        nc.scalar.dma_start(out=x_sb[:, 0, 6:8, :], in_=xT_r[:, 0, 6:8, :])
        nc.gpsimd.dma_start(out=w_sb[:, 2:4, :], in_=wT_r[:, 2:4, :])
        nc.sync.dma_start(out=w_sb[:, 4:6, :], in_=wT_r[:, 4:6, :])
        nc.scalar.dma_start(out=w_sb[:, 6:8, :], in_=wT_r[:, 6:8, :])
        for s in range(1, NS):
            nc.sync.dma_start(out=x_sb[:, s, 0:4, :], in_=xT_r[:, s, 0:4, :])
            nc.scalar.dma_start(out=x_sb[:, s, 4:8, :], in_=xT_r[:, s, 4:8, :])
        nc.gpsimd.dma_start(out=msk_sb, in_=msk_d[:, :, :, :, :])

        out_r = out_d.rearrange("(nb p) o -> p nb o", p=128)

        def emit_proj_m(s, m):
            # one q/k chain, transposed [o_part, t]; m-tiles: q0 q1 k0 k1
            ps = pj_ps.tile([128, TS], f32, tag="ps")
            for c in range(CC):
                nc.tensor.matmul(
                    ps,
                    lhsT=w_sb[:, c, m * 128:(m + 1) * 128],
                    rhs=x_sb[:, s, c, :],
                    start=(c == 0),
                    stop=(c == CC - 1),
                )
            dst = (q_sb if m < 2 else k_sb)[:, m % 2, s * TS:(s + 1) * TS]
            nc.scalar.copy(dst, ps)

        def emit_proj_v(s, t4):
            # one v chain, [t_part, o]
            pv = pj_ps.tile([128, TS], f32, tag="ps")
            for c in range(CC):
                nc.tensor.matmul(
                    pv[:, 0:D * HPC],
                    lhsT=x_sb[:, s, c, t4 * 128:(t4 + 1) * 128],
                    rhs=w_sb[:, c, 2 * D * HPC:3 * D * HPC],
                    start=(c == 0),
                    stop=(c == CC - 1),
                )
            tb = s * (TS // 128) + t4
            nc.vector.tensor_copy(
                v_sb[:, tb, :, 0:D],
                pv[:, 0:D * HPC].rearrange("p (h d) -> p h d", h=HPC),
            )

        def emit_qk(j):
            qlo = max(0, j - 1)
            qhi = min(NB - 1, j + 1)
            nq = (qhi - qlo + 1) * 128
            for mt in range(2):
                sct = sc_ps.tile([128, 2, TS], f32, tag="sc")
                for hh in range(2):
                    po = hh * 64
                    # scores^T chunk: [key j (part), query window (free)]
                    nc.tensor.matmul(
                        sct[:, hh, 0:nq],
                        lhsT=k_sb[po:po + 64, mt, j * 128:(j + 1) * 128],
                        rhs=q_sb[po:po + 64, mt, qlo * 128:(qhi + 1) * 128],
                        start=True,
                        stop=True,
                    )
                nc.scalar.activation(
                    p_sb[:, mt, j, :, 0:nq], sct[:, :, 0:nq], Exp
                )
            if j >= 1:  # query block j-1 sees chunk j as "next" (mask type 1)
                pm = p_sb[:, :, j, :, 0:128]
                nc.vector.tensor_mul(pm, pm, msk_sb[:, 1])
            if j <= NB - 2:  # query block j+1 sees chunk j as "prev" (type 0)
                c0 = (j + 1 - qlo) * 128
                pm = p_sb[:, :, j, :, c0:c0 + 128]
                nc.vector.tensor_mul(pm, pm, msk_sb[:, 0])

        def emit_pv(i):
            jbs = [jb for jb in (i - 1, i, i + 1) if 0 <= jb < NB]
            ov = ov_ps.tile([128, HPC, D + 1], f32, tag="ov")
            for h in range(HPC):
                mt, hh = divmod(h, 2)
                for n, j in enumerate(jbs):
                    ci = i - max(0, j - 1)
                    nc.tensor.matmul(
                        ov[:, h, :],
                        lhsT=p_sb[:, mt, j, hh, ci * 128:(ci + 1) * 128],
                        rhs=v_sb[:, j, h, :],
                        start=(n == 0),
                        stop=(n == len(jbs) - 1),
                    )
            r_t = lpool.tile([128, HPC], f32, tag="r")
            nc.vector.reciprocal(r_t, ov[:, :, D])
            for h in range(HPC):
                nc.vector.tensor_scalar_mul(
                    o_sb[:, i, h * D:(h + 1) * D], ov[:, h, 0:D], r_t[:, h:h + 1]
                )
            # batched early output DMAs; small late ones so the drain after
            # the last norm is ~one 128KB transfer, not a 10us dribble
            out_grp = {
                3: (nc.sync, 0, 4), 7: (nc.gpsimd, 4, 4), 11: (nc.gpsimd, 8, 4),
                13: (nc.sync, 12, 2), 14: (nc.scalar, 14, 1), 15: (nc.sync, 15, 1),
            }
            if i in out_grp:
                eng, b0, nb = out_grp[i]
                eng.dma_start(
                    out=out_r[:, b0:b0 + nb, :], in_=o_sb[:, b0:b0 + nb, :]
                )

        # fine interleave: attention units are woven between individual
        # projection chains so the in-order tensor queue never stalls on the
        # scalar exp chain — a not-yet-ready attention op would otherwise
        # block ready projection matmuls queued behind it.
        # s=0: prime the pipeline
        for m in range(4):
            emit_proj_m(0, m)
        emit_qk(0)
        emit_qk(1)
        emit_qk(2)
        emit_proj_v(0, 0)
        emit_proj_v(0, 1)
        emit_pv(0)
        emit_proj_v(0, 2)
        emit_pv(1)
        emit_proj_v(0, 3)
        for s in range(1, NS):
            emit_proj_m(s, 0)
            emit_proj_m(s, 1)
            emit_qk(4 * s - 1)
            emit_proj_m(s, 2)
            emit_proj_m(s, 3)
            emit_qk(4 * s)
            emit_qk(4 * s + 1)
            emit_qk(4 * s + 2)
            if s == NS - 1:
                emit_qk(4 * s + 3)
            emit_proj_v(s, 0)
            emit_pv(4 * s - 2)
            emit_proj_v(s, 1)
            emit_pv(4 * s - 1)
            emit_proj_v(s, 2)
            emit_pv(4 * s)
            emit_proj_v(s, 3)
            emit_pv(4 * s + 1)
        emit_pv(NB - 2)
        emit_pv(NB - 1)

    nc.compile()
    return nc


def _host_inputs(x, Wqkv):
    """Per-core input maps: shard batch x head-group, bf16, device layouts."""
    import ml_dtypes

    bf = ml_dtypes.bfloat16
    scale = float(D) ** -0.5
    r = np.arange(128, dtype=np.float32)[:, None]
    ci = np.arange(128, dtype=np.float32)[None, :]
    # type 0 (chunk is "prev" of the query block): allowed iff c <= r
    # type 1 (chunk is "next" of the query block): allowed iff c >= r
    msk2 = np.stack([(ci <= r), (ci >= r)], axis=1).astype(np.float32)
    msk = np.ascontiguousarray(
        np.broadcast_to(msk2[:, :, None, None, :], (128, 2, 2, 2, 128))
    ).astype(bf)

    x = np.asarray(x, dtype=np.float32)
    Wqkv = np.asarray(Wqkv, dtype=np.float32)
    # [p, s, cc, t] device layout, per-partition contiguous
    xT = [
        np.ascontiguousarray(
            x[b].T.reshape(CC, 128, NS, TS).transpose(1, 2, 0, 3).reshape(128, -1)
        ).astype(bf)
        for b in range(B)
    ]
    in_maps = []
    for core in range(N_CORES):
        b, hg = divmod(core, N_CORES // B)
        rows = slice(hg * HPC * D, (hg + 1) * HPC * D)
        wcat = np.concatenate(
            [
                Wqkv[0 * C:1 * C][rows] * scale,
                Wqkv[1 * C:2 * C][rows],
                Wqkv[2 * C:3 * C][rows],
            ],
            axis=0,
        )
        # [p, cc, o] device layout
        w = np.ascontiguousarray(
            wcat.T.reshape(CC, 128, OPC).transpose(1, 0, 2).reshape(128, -1)
        ).astype(bf)
        in_maps.append({"xT": xT[b], "wT": w, "msk": msk})
    return in_maps


def _gather(results):
    out = np.empty((B, T, C), dtype=np.float32)
    for core in range(N_CORES):
        b, hg = divmod(core, N_CORES // B)
        out[b, :, hg * HPC * D:(hg + 1) * HPC * D] = results[core]["out"]
    return out


def kernel(x, Wqkv):
    from concourse.bass_utils import run_bass_kernel_spmd

    key = PDT_NAME
    if key not in _PROGRAM_CACHE:
        _PROGRAM_CACHE[key] = _build_program(key)
    nc = _PROGRAM_CACHE[key]
    in_maps = _host_inputs(x, Wqkv)
    res = run_bass_kernel_spmd(nc, in_maps, list(range(N_CORES)))
    return _gather(res.results)


# revision 24
# speedup vs baseline: 1.0573x; 1.0190x over previous
"""Trainium2 Bass kernel for nn_BaseAttention (sliding-window attention).

Full-input contract: kernel(x, Wqkv) -> [B, T, C] float32.

Sharding (8 cores): data-parallel over B (2) x tensor-parallel over head
groups (16 heads -> 4 groups of 4). Core c handles batch c//4, head group
c%4. Each core computes its QKV projection slice (768 of 3072 output rows)
and banded attention for its 4 heads; outputs are disjoint channel slices
of the final [B, T, C] tensor, so no collectives are needed.

All matmul inputs are bf16 (rel err ~8e-3 vs the 2e-2 gate); accumulation
stays fp32 in PSUM. Design notes (from trace analysis):
  - per-matmul tensor cost ~= out_rows * 0.42ns when dependencies are hot,
    so instruction shapes maximize the moving dim: projection q,k in
    transposed [d, t] layout (512-wide), v in [t, d] (256-wide).
  - attention is key-chunk-centric: for key chunk j, ONE matmul produces
    scores^T [key 128, query <=384] covering query blocks j-1..j+1; exp is
    batched over head pairs on the scalar engine; the two window-mask
    multiplies are batched over all 4 heads on the (otherwise idle) gpsimd
    engine; PV is query-centric, accumulating [q, d|ones] chunks in PSUM so
    the softmax denominator falls out of the same matmuls; the vector
    engine normalizes straight out of PSUM. No PE transposes anywhere.
  - host pre-arranges x/W so every DMA is per-partition contiguous; x
    arrives in t-slices so the first projection matmul starts ~3us after
    the DMA queues open.
"""

import sys

import numpy as np

if "/opt/trn_rl_repo" not in sys.path:
    sys.path.insert(0, "/opt/trn_rl_repo")

B, T, C = 2, 2048, 1024
HEADS = 16
D = C // HEADS  # 64
WINDOW = 128
N_CORES = 8
HPC = HEADS // 4  # heads per core (4)
OPC = 3 * HPC * D  # projection output rows per core (768)

PDT_NAME = "bf16"

_PROGRAM_CACHE = {}

CC = C // 128  # 8 contraction chunks
TS = 512  # projection t-slice
NS = T // TS  # 4 slices
NB = T // 128  # 16 query / key blocks


def _build_program(pdt_name="bf16"):
    import concourse.mybir as mybir
    from concourse import bacc
    import concourse.tile as tile
    from contextlib import ExitStack

    f32 = mybir.dt.float32
    bf16 = mybir.dt.bfloat16
    Exp = mybir.ActivationFunctionType.Exp
    Ident = mybir.ActivationFunctionType.Identity

    nc = bacc.Bacc()
    # host pre-arranged, per-partition contiguous layouts
    xT_d = nc.declare_dram_parameter("xT", [128, NS * CC * TS], bf16, isOutput=False)
    wT_d = nc.declare_dram_parameter("wT", [128, CC * OPC], bf16, isOutput=False)
    msk_d = nc.declare_dram_parameter("msk", [128, 2, 2, 2, 128], bf16, isOutput=False)
    out_d = nc.declare_dram_parameter("out", [T, HPC * D], f32, isOutput=True)

    with ExitStack() as ctx:
        tc = ctx.enter_context(tile.TileContext(nc))
        const = ctx.enter_context(tc.tile_pool(name="const", bufs=1))
        lpool = ctx.enter_context(tc.tile_pool(name="lp", bufs=8))
        pj_ps = ctx.enter_context(tc.tile_pool(name="pjps", bufs=2, space="PSUM"))
        sc_ps = ctx.enter_context(tc.tile_pool(name="scps", bufs=2, space="PSUM"))
        ov_ps = ctx.enter_context(tc.tile_pool(name="ovps", bufs=2, space="PSUM"))

        w_sb = const.tile([128, CC, OPC], bf16)
        x_sb = const.tile([128, NS, CC, TS], bf16)
        msk_sb = const.tile([128, 2, 2, 2, 128], bf16)
        q_sb = const.tile([128, 2, T], bf16)
        k_sb = const.tile([128, 2, T], bf16)
        # v packed per (key block, head) with a trailing ones column: the PV
        # matmul emits the output block and the softmax denominator at once.
        v_sb = const.tile([128, NB, HPC, D + 1], bf16)
        # exp'd transposed scores, [p, mt, j, hh, 3*128]
        p_sb = const.tile([128, 2, NB, 2, 3 * 128], bf16)
        o_sb = const.tile([128, NB, HPC * D], f32)
        nc.vector.memset(v_sb[:, :, :, D:D + 1], 1.0)

        xT_r = xT_d.rearrange("p (s cc t) -> p s cc t", s=NS, cc=CC)
        wT_r = wT_d.rearrange("p (cc o) -> p cc o", cc=CC)
        # spread early-needed tensors over all three DMA-capable queues
        # (gpsimd/sync/scalar, each ~128GB/s), finely chunked up front so the
        # first projection chain starts as soon as w[cc0]+x[s0,cc0] land.
        nc.gpsimd.dma_start(out=w_sb[:, 0:1, :], in_=wT_r[:, 0:1, :])
        nc.sync.dma_start(out=x_sb[:, 0, 0:2, :], in_=xT_r[:, 0, 0:2, :])
        nc.scalar.dma_start(out=x_sb[:, 0, 4:6, :], in_=xT_r[:, 0, 4:6, :])
        nc.gpsimd.dma_start(out=w_sb[:, 1:2, :], in_=wT_r[:, 1:2, :])
        nc.sync.dma_start(out=x_sb[:, 0, 2:4, :], in_=xT_r[:, 0, 2:4, :])
        nc.scalar.dma_start(out=x_sb[:, 0, 6:8, :], in_=xT_r[:, 0, 6:8, :])
        nc.gpsimd.dma_start(out=w_sb[:, 2:4, :], in_=wT_r[:, 2:4, :])
        nc.sync.dma_start(out=w_sb[:, 4:6, :], in_=wT_r[:, 4:6, :])
        nc.scalar.dma_start(out=w_sb[:, 6:8, :], in_=wT_r[:, 6:8, :])
        for s in range(1, NS):
            nc.sync.dma_start(out=x_sb[:, s, 0:4, :], in_=xT_r[:, s, 0:4, :])
            nc.scalar.dma_start(out=x_sb[:, s, 4:8, :], in_=xT_r[:, s, 4:8, :])
        nc.gpsimd.dma_start(out=msk_sb, in_=msk_d[:, :, :, :, :])

        out_r = out_d.rearrange("(nb p) o -> p nb o", p=128)

        def emit_proj_m(s, m):
            # one q/k chain, transposed [o_part, t]; m-tiles: q0 q1 k0 k1
            ps = pj_ps.tile([128, TS], f32, tag="ps")
            for c in range(CC):
                nc.tensor.matmul(
                    ps,
                    lhsT=w_sb[:, c, m * 128:(m + 1) * 128],
                    rhs=x_sb[:, s, c, :],
                    start=(c == 0),
                    stop=(c == CC - 1),
                )
            dst = (q_sb if m < 2 else k_sb)[:, m % 2, s * TS:(s + 1) * TS]
            nc.scalar.copy(dst, ps)

        def emit_proj_v(s, t4):
            # one v chain, [t_part, o]
            pv = pj_ps.tile([128, TS], f32, tag="ps")
            for c in range(CC):
                nc.tensor.matmul(
                    pv[:, 0:D * HPC],
                    lhsT=x_sb[:, s, c, t4 * 128:(t4 + 1) * 128],
                    rhs=w_sb[:, c, 2 * D * HPC:3 * D * HPC],
                    start=(c == 0),
                    stop=(c == CC - 1),
                )
            tb = s * (TS // 128) + t4
            nc.vector.tensor_copy(
                v_sb[:, tb, :, 0:D],
                pv[:, 0:D * HPC].rearrange("p (h d) -> p h d", h=HPC),
            )

        def emit_qk(j):
            qlo = max(0, j - 1)
            qhi = min(NB - 1, j + 1)
            nq = (qhi - qlo + 1) * 128
            for mt in range(2):
                sct = sc_ps.tile([128, 2, TS], f32, tag="sc")
                for hh in range(2):
                    po = hh * 64
                    # scores^T chunk: [key j (part), query window (free)]
                    nc.tensor.matmul(
                        sct[:, hh, 0:nq],
                        lhsT=k_sb[po:po + 64, mt, j * 128:(j + 1) * 128],
                        rhs=q_sb[po:po + 64, mt, qlo * 128:(qhi + 1) * 128],
                        start=True,
                        stop=True,
                    )
                nc.scalar.activation(
                    p_sb[:, mt, j, :, 0:nq], sct[:, :, 0:nq], Exp
                )
            if j >= 1:  # query block j-1 sees chunk j as "next" (mask type 1)
                pm = p_sb[:, :, j, :, 0:128]
                nc.vector.tensor_mul(pm, pm, msk_sb[:, 1])
            if j <= NB - 2:  # query block j+1 sees chunk j as "prev" (type 0)
                c0 = (j + 1 - qlo) * 128
                pm = p_sb[:, :, j, :, c0:c0 + 128]
                nc.vector.tensor_mul(pm, pm, msk_sb[:, 0])

        def emit_pv(i):
            jbs = [jb for jb in (i - 1, i, i + 1) if 0 <= jb < NB]
            ov = ov_ps.tile([128, HPC, D + 1], f32, tag="ov")
            for h in range(HPC):
                mt, hh = divmod(h, 2)
                for n, j in enumerate(jbs):
                    ci = i - max(0, j - 1)
                    nc.tensor.matmul(
                        ov[:, h, :],
                        lhsT=p_sb[:, mt, j, hh, ci * 128:(ci + 1) * 128],
                        rhs=v_sb[:, j, h, :],
                        start=(n == 0),
                        stop=(n == len(jbs) - 1),
                    )
            r_t = lpool.tile([128, HPC], f32, tag="r")
            nc.vector.reciprocal(r_t, ov[:, :, D])
            nc.vector.tensor_mul(
                o_sb[:, i, :].rearrange("p (h d) -> p h d", h=HPC),
                ov[:, :, 0:D],
                r_t[:, :].unsqueeze(2).broadcast_to([128, HPC, D]),
            )
            # batched early output DMAs; small late ones so the drain after
            # the last norm is ~one 128KB transfer, not a 10us dribble
            out_grp = {
                3: (nc.sync, 0, 4), 7: (nc.gpsimd, 4, 4), 11: (nc.gpsimd, 8, 4),
                13: (nc.sync, 12, 2), 14: (nc.scalar, 14, 1), 15: (nc.sync, 15, 1),
            }
            if i in out_grp:
                eng, b0, nb = out_grp[i]
                eng.dma_start(
                    out=out_r[:, b0:b0 + nb, :], in_=o_sb[:, b0:b0 + nb, :]
                )

        # fine interleave: attention units are woven between individual
        # projection chains so the in-order tensor queue never stalls on the
        # scalar exp chain — a not-yet-ready attention op would otherwise
        # block ready projection matmuls queued behind it.
        # s=0: prime the pipeline
        for m in range(4):
            emit_proj_m(0, m)
        emit_qk(0)
        emit_qk(1)
        emit_qk(2)
        emit_proj_v(0, 0)
        emit_proj_v(0, 1)
        emit_pv(0)
        emit_proj_v(0, 2)
        emit_pv(1)
        emit_proj_v(0, 3)
        for s in range(1, NS):
            emit_proj_m(s, 0)
            emit_proj_m(s, 1)
            emit_qk(4 * s - 1)
            emit_proj_m(s, 2)
            emit_proj_m(s, 3)
            emit_qk(4 * s)
            emit_qk(4 * s + 1)
            emit_qk(4 * s + 2)
            if s == NS - 1:
                emit_qk(4 * s + 3)
            emit_proj_v(s, 0)
            emit_pv(4 * s - 2)
            emit_proj_v(s, 1)
            emit_pv(4 * s - 1)
            emit_proj_v(s, 2)
            emit_pv(4 * s)
            emit_proj_v(s, 3)
            emit_pv(4 * s + 1)
        emit_pv(NB - 2)
        emit_pv(NB - 1)

    nc.compile()
    return nc


def _host_inputs(x, Wqkv):
    """Per-core input maps: shard batch x head-group, bf16, device layouts."""
    import ml_dtypes

    bf = ml_dtypes.bfloat16
    scale = float(D) ** -0.5
    r = np.arange(128, dtype=np.float32)[:, None]
    ci = np.arange(128, dtype=np.float32)[None, :]
    # type 0 (chunk is "prev" of the query block): allowed iff c <= r
    # type 1 (chunk is "next" of the query block): allowed iff c >= r
    msk2 = np.stack([(ci <= r), (ci >= r)], axis=1).astype(np.float32)
    msk = np.ascontiguousarray(
        np.broadcast_to(msk2[:, :, None, None, :], (128, 2, 2, 2, 128))
    ).astype(bf)

    x = np.asarray(x, dtype=np.float32)
    Wqkv = np.asarray(Wqkv, dtype=np.float32)
    # [p, s, cc, t] device layout, per-partition contiguous
    xT = [
        np.ascontiguousarray(
            x[b].T.reshape(CC, 128, NS, TS).transpose(1, 2, 0, 3).reshape(128, -1)
        ).astype(bf)
        for b in range(B)
    ]
    in_maps = []
    for core in range(N_CORES):
        b, hg = divmod(core, N_CORES // B)
        rows = slice(hg * HPC * D, (hg + 1) * HPC * D)
        wcat = np.concatenate(
            [
                Wqkv[0 * C:1 * C][rows] * scale,
                Wqkv[1 * C:2 * C][rows],
                Wqkv[2 * C:3 * C][rows],
            ],
            axis=0,
        )
        # [p, cc, o] device layout
        w = np.ascontiguousarray(
            wcat.T.reshape(CC, 128, OPC).transpose(1, 0, 2).reshape(128, -1)
        ).astype(bf)
        in_maps.append({"xT": xT[b], "wT": w, "msk": msk})
    return in_maps


def _gather(results):
    out = np.empty((B, T, C), dtype=np.float32)
    for core in range(N_CORES):
        b, hg = divmod(core, N_CORES // B)
        out[b, :, hg * HPC * D:(hg + 1) * HPC * D] = results[core]["out"]
    return out


def kernel(x, Wqkv):
    from concourse.bass_utils import run_bass_kernel_spmd

    key = PDT_NAME
    if key not in _PROGRAM_CACHE:
        _PROGRAM_CACHE[key] = _build_program(key)
    nc = _PROGRAM_CACHE[key]
    in_maps = _host_inputs(x, Wqkv)
    res = run_bass_kernel_spmd(nc, in_maps, list(range(N_CORES)))
    return _gather(res.results)
